# revision 15
# baseline (speedup 1.0000x reference)
"""FitzHugh-Nagumo Euler recurrence kernel for Trainium2 (8 NeuronCores).

Problem: z (16, 2000, 1024) f32 -> V (16, 2000, 1024) f32 where
    v[k+1] = (1+dt) v[k] - (dt/3) v[k]^3 - dt s[k] + dt z[k]
    s[k+1] = s[k] + dt*0.08*(v[k] + 0.7 - 0.8 s[k])
    out[:, k] = v[k] / 2,  v[0] = s[0] = 0.

Strategy:
  - Pure data parallel over the B*L = 16384 independent lanes: 2 batches
    per core x 8 cores -> 2048 lanes/core = 128 partitions x 16 free.
  - First-order rewrite in u = v/2 with a combined slow state
    R~ = (W - K2 - zd)/(dt*beta), W = (dt/2)s, which absorbs both the
    recovery variable s and the z-forcing, so each Euler step is exactly
    3 fused DVE instructions (M, U, R) whose producer->consumer
    distances are all >= 2 (avoids the DVE read-write turnaround bubble
    on back-to-back dependent ops):
        S~[k]  = 62.5*(alpha z[k] - z[k+1]) + gamma/(2 beta)   (bulk:
                 ACT scale, Pool subtract, ACT scale+bias - off the DVE)
        opM[k]: m~[k]  = u[k] + S~[k]                     (DVE tensor_tensor)
        opU[k]: u[k+1] = 1.1 u[k] - 4cg u[k]^3 - dtb R~[k]  (custom DVE op)
        opR[k]: R~[k+1] = alpha R~[k] + m~[k]             (DVE stt)
    u_0 = 0, R~_0 = -62.5 z_0. Exactly equivalent to the reference
    recurrence; carries the slow s-state explicitly so fp32 roundoff is
    not amplified (an s-eliminating second-order form loses ~100x
    accuracy). 3 DVE instructions per Euler step is the floor: the R~
    update has 3 independent tensor inputs and DVE ops take only 2.
"""

import numpy as np

DT = 0.1
B, T, L = 16, 2000, 1024
NCORES = 8
P = 128                      # SBUF partitions
FD = (2 * L) // P            # 16 lanes per partition per core
TC = 250                     # timesteps per chunk
NCH = T // TC                # 8 chunks

ALPHA = 1.0 - DT * 0.08 * 0.8          # s-decay       0.9936
BETA = DT * 0.08                       # s<-v coupling 0.008
GAMMA = DT * 0.08 * 0.7                # s const       0.0056
C1 = (1.0 + DT) + ALPHA                # 2.0936
C2 = (1.0 + DT) * ALPHA + DT * BETA    # 1.09376
CG = DT / 3.0

# u = v/2 scaled coefficients (first-order scheme)
K2 = (DT / 2.0) * GAMMA / (1.0 - ALPHA)  # 0.04375
DTB = DT * BETA                          # 0.0008
Q0 = -K2 / DTB                           # -54.6875
OPA_C0 = 1.0 + DT          # * u_k
OPA_C1 = -4.0 * CG         # * u_k^3
OPA_C2 = -DTB              # * R~_k
ZD_SCALE = DT / 2.0
ZD_BIAS = -K2
INV2B = 1.0 / (2.0 * BETA)   # 62.5
SB = GAMMA / (2.0 * BETA)    # 0.35 (S~ bias)

_CACHE = {}


def _register_ops():
    """Runtime-register the two fused FHN custom DVE ops."""
    from concourse.dve_spec import (
        Spec, Src0, Src1, C0 as C0L, C1 as C1L, C2 as C2L, sq, lower,
    )
    from concourse.dve_uop import DveOpSpec
    import concourse.dve_ops as dve_ops
    from concourse.dve_ops import DveOp, OPS

    def make_op(name, spec):
        if name in dve_ops._SUB_OPCODE_FOR_NAME:
            for op in OPS:
                if op.name == name:
                    return op
        row = 1 + len(OPS)
        assert row < 0x20
        shas = {}
        for ver in ("v3", "v4"):
            s = DveOpSpec(name=name, opcode=row, uops=lower(spec, ver=ver), rd1_en=True)
            shas[ver] = s.sha(ver)
        op = DveOp(name, spec, subdim=False, uops_sha=shas)
        OPS.append(op)
        dve_ops._SUB_OPCODE_FOR_NAME[name] = row
        dve_ops.CUSTOM_DVE_SPECS[name] = spec
        return op

    # u' = (u*C0 + R*C2) + u^2*(u*C1) — this exact fp32 association is
    # load-bearing: the recurrence amplifies per-step rounding ~1e3x over
    # T=2000 steps, and this tree gives the smallest end-to-end deviation
    # from the fp32 jax reference among the equivalent associations.
    fh_u = make_op(
        "FH_U7_ANT",
        Spec(
            body=(Src0 * C0L + Src1 * C2L) + sq(Src0) * (Src0 * C1L),
            reference=lambda in0, in1, s0, s1, imm2: (
                (in0 * s0 + in1 * imm2) + (in0 * in0) * (in0 * np.float32(s1))
            ).astype(np.float32),
        ),
    )
    return fh_u


def _build_program():
    import concourse.bacc as bacc
    import concourse.mybir as mybir
    from concourse.tile import TileContext

    fh_u = _register_ops()
    f32 = mybir.dt.float32
    au = mybir.AluOpType

    nc = bacc.Bacc("TRN2", target_bir_lowering=False, debug=False)
    z_d = nc.dram_tensor("z", [P, T * FD], f32, kind="ExternalInput")
    v_d = nc.dram_tensor("v", [P, T * FD], f32, kind="ExternalOutput")
    z_ap = z_d.ap()
    v_ap = v_d.ap()

    Copy = mybir.ActivationFunctionType.Copy
    with TileContext(nc) as tc:
        with (
            tc.tile_pool(name="zp", bufs=2) as zp,
            tc.tile_pool(name="wp", bufs=2) as wp,
            tc.tile_pool(name="stp", bufs=3) as stp,
            tc.tile_pool(name="vp", bufs=3) as vp,
            tc.tile_pool(name="small", bufs=1) as sp,
        ):
            r_t = [
                sp.tile([P, FD], f32, tag="r0", name="r0"),
                sp.tile([P, FD], f32, tag="r1", name="r1"),
            ]
            m_t = [
                sp.tile([P, FD], f32, tag="m0", name="m0"),
                sp.tile([P, FD], f32, tag="m1", name="m1"),
            ]
            # 0.35-filled const tile for the S~ bias (ACT's fused
            # scale+bias Copy is NOT bit-exact fp32; scale-only is, so the
            # bias is added separately on the Pool engine).
            c35_t = sp.tile([P, TC * FD], f32, tag="c35", name="c35")
            nc.gpsimd.memset(c35_t[:], SB)
            z_tiles, s_tiles, v_tiles = {}, {}, {}

            def fetch_chunk(c):
                """Emit Z-DMA + bulk S~ prep (ACT + Pool, off the DVE)."""
                if c in z_tiles or c >= NCH:
                    return
                zcols = min(TC + 1, T - c * TC)
                zt = zp.tile([P, zcols * FD], f32, tag="z", name="zt")
                nc.sync.dma_start(
                    zt[:], z_ap[:, c * TC * FD : (c * TC + zcols) * FD]
                )
                z_tiles[c] = zt
                m = min(TC, (T - 2) - c * TC)  # S~ entries in this chunk
                wt = wp.tile([P, TC * FD], f32, tag="w", name="wt")
                st = stp.tile([P, TC * FD], f32, tag="s", name="st")
                # w = alpha * z_k  (ACT, scale-only: bit-exact)
                nc.scalar.activation(
                    wt[:, : m * FD], zt[:, : m * FD], Copy,
                    scale=float(ALPHA),
                )
                # s = w - z_{k+1}  (Pool)
                nc.gpsimd.tensor_tensor(
                    out=st[:, : m * FD], in0=wt[:, : m * FD],
                    in1=zt[:, FD : (m + 1) * FD], op=au.subtract,
                )
                # s = 62.5 * s  (ACT, scale-only, in place)
                nc.scalar.activation(
                    st[:, : m * FD], st[:, : m * FD], Copy,
                    scale=float(INV2B),
                )
                # S~ = s + 0.35  (Pool, const tile)
                nc.gpsimd.tensor_tensor(
                    out=st[:, : m * FD], in0=st[:, : m * FD],
                    in1=c35_t[:, : m * FD], op=au.add,
                )
                s_tiles[c] = st

            def v_tile(c):
                if c not in v_tiles:
                    v_tiles[c] = vp.tile([P, TC * FD], f32, tag="v", name="vt")
                return v_tiles[c]

            def vcol(k):
                return v_tile(k // TC)[:, (k % TC) * FD : (k % TC + 1) * FD]

            # chunk 0 + prefetch chunk 1
            fetch_chunk(0)
            fetch_chunk(1)

            # u_0 = 0 ; R~_0 = -62.5 z_0 (ACT)
            nc.vector.memset(vcol(0), 0.0)
            nc.scalar.activation(
                r_t[0][:], z_tiles[0][:, 0:FD], Copy, scale=float(-INV2B)
            )

            for k in range(T - 1):
                cj, oj = k // TC, k % TC
                if oj == 0 and k > 0:
                    fetch_chunk(cj + 1)  # prefetch next chunk's z/S~
                if k <= T - 3:
                    # m~ = u_k + S~_k
                    nc.vector.tensor_tensor(
                        out=m_t[k % 2][:], in0=vcol(k),
                        in1=s_tiles[cj][:, oj * FD : (oj + 1) * FD], op=au.add,
                    )
                # u' = 1.1 u - 4cg u^3 - dtb R~
                nc.vector._custom_dve(
                    fh_u, out=vcol(k + 1), in0=vcol(k), in1=r_t[k % 2][:],
                    s0=OPA_C0, s1=OPA_C1, imm2=OPA_C2,
                )
                if k <= T - 3:
                    # R~' = alpha R~ + m~
                    nc.vector.scalar_tensor_tensor(
                        r_t[(k + 1) % 2][:], r_t[k % 2][:], ALPHA, m_t[k % 2][:],
                        op0=au.mult, op1=au.add,
                    )
                if (k + 1) % TC == TC - 1:
                    # chunk of col k+1 is complete -> DMA out
                    cc = (k + 1) // TC
                    nc.sync.dma_start(
                        v_ap[:, cc * TC * FD : (cc + 1) * TC * FD], v_tiles[cc][:]
                    )
    nc.compile()
    return nc


def _get_program():
    if "nc" not in _CACHE:
        _CACHE["nc"] = _build_program()
    return _CACHE["nc"]


def _shard_input(z):
    """z (B,T,L) -> list of 8 per-core arrays (P, T*FD), lane-major layout."""
    shards = []
    for c in range(NCORES):
        zc = z[2 * c : 2 * c + 2]                      # (2, T, L)
        arr = zc.transpose(0, 2, 1).reshape(2 * L, T)  # (lane, T)
        arr = arr.reshape(P, FD, T).transpose(0, 2, 1) # (P, T, FD)
        shards.append(np.ascontiguousarray(arr, dtype=np.float32).reshape(P, T * FD))
    return shards


def _unshard_output(outs):
    """list of 8 (P, T*FD) -> (B, T, L)."""
    full = np.empty((B, T, L), dtype=np.float32)
    for c, o in enumerate(outs):
        arr = o.reshape(P, T, FD).transpose(0, 2, 1).reshape(2 * L, T)
        full[2 * c : 2 * c + 2] = arr.reshape(2, L, T).transpose(0, 2, 1)
    return full


def kernel(z, _trace=False):
    from concourse.bass_utils import run_bass_kernel_spmd

    z = np.asarray(z, dtype=np.float32)
    assert z.shape == (B, T, L), z.shape
    nc = _get_program()
    in_maps = [{"z": s} for s in _shard_input(z)]
    res = run_bass_kernel_spmd(
        nc, in_maps, core_ids=list(range(NCORES)), trace=_trace
    )
    out = _unshard_output([r["v"] for r in res.results])
    if _trace:
        _CACHE["last_results"] = res
    return out
